# revision 31
# baseline (speedup 1.0000x reference)
"""Bass/Trainium2 kernel for BilinearlyModulatedAttention (v2, bf16).

Sharding: 8 cores = 2 (batch) x 4 (head groups of 4 heads).
Each core computes, for its batch b and heads [4g, 4g+4):
  QT/KT (feature-major, d x T), V (token-major), bilinear gate, causal
  softmax in transposed layout (scores[s, t]), PV with a ones-column
  giving softmax denominators, normalization, and a partial output
  projection Y_partial = O^T.T @ W_out[rows]. Host sums the 4 partials
  per batch and adds b_out.

v2 changes vs the fp32r baseline (283-297us):
 - ALL matmul operands in bf16 (fp32r measured ~2 cycles/row on HW at
   N=512 warm: 428ns avg; bf16 streams 1 cycle/row and enables FWL
   fast weight loads). PSUM accumulation stays fp32.
 - scores for the two heads of a pair go into ONE 2-bank psum tile
   ([128,2,512]: bank0 = row-group-0 head, bank1 = row-group-64 head,
   satisfying the one-row-group-base-per-bank rule), so exp is ONE
   ACT instruction over both heads (halves ACT instruction count).
 - input DMAs coalesced (~20 instead of ~40) and split across the
   sync + gpsimd queues; x arrives per-k chunk-pair so the first
   matmul starts ~3us in (baseline was DMA-starved for 40us).
 - psum: scores pool [128,2,512]x2 (4 banks) reserved for b_iters,
   filler pool x1 (2 banks) for qk/vg/out-proj jobs, U pool [65,2,512]
   x1 (2 banks). 8 banks total.
 - softmax denominator row copies go through ACT (Copy) instead of DVE;
   normalize DMA bounces ride the gpsimd queue.
 - out-projection does both 512-col slabs per token tile in one job
   (one DVE drain + one y DMA per 128 tokens).
"""

import sys

if "/opt/trn_rl_repo" not in sys.path:
    sys.path.insert(0, "/opt/trn_rl_repo")

import numpy as np

D_MODEL = 1024
N_HEADS = 16
D_HEAD = 64
B = 2
T_FULL = 2048
N_CORES = 8
H_LOC = N_HEADS // (N_CORES // B)  # 4 heads per core

_LDW_PATCHED = False


def _patch_ldw_opt():
    """Compile walrus with --enable-ldw-opt=true (elides redundant
    LDWEIGHTS reloads). Wraps concourse.bass_utils.run_command."""
    global _LDW_PATCHED
    if _LDW_PATCHED:
        return
    import concourse.bass_utils as BU
    orig = BU.run_command

    def run_patched(argv, **kw):
        argv = [a.replace("--enable-ldw-opt=false", "--enable-ldw-opt=true")
                if isinstance(a, str) else a for a in argv]
        return orig(argv, **kw)

    BU.run_command = run_patched
    _LDW_PATCHED = True


def build_nc(T=T_FULL, D=D_MODEL, h_loc=H_LOC, dh=D_HEAD, W=512):
    """Build the Bass module for one core's shard. Returns (nc, meta)."""
    import concourse.bass as bass
    import concourse.mybir as mybir
    import concourse.tile as tile
    from concourse import bacc
    from contextlib import ExitStack
    from collections import deque

    f32 = mybir.dt.float32
    bf = mybir.dt.bfloat16
    AF = mybir.ActivationFunctionType
    ALU = mybir.AluOpType

    KN = D // 128             # k-tiles for the qkv projections
    TT = T // 128             # 128-token tiles
    assert T % W == 0 and W == 512
    NCH = T // W              # chunks
    CPW = min(2, NCH)         # chunks per qk/projection job
    NCP = (NCH + CPW - 1) // CPW  # chunk-pair jobs
    W128 = W // 128           # s-tiles per chunk (4)
    DHL = h_loc * dh          # local head dim total (256)
    NP = h_loc // 2           # head pairs
    KO = DHL // 128           # out-proj k-tiles (2)
    VGW = dh + 1              # vg last dim: 64 V cols + ones col
    SCALE = 1.0 / float(np.sqrt(dh))

    nc = bacc.Bacc("TRN2", target_bir_lowering=False, debug=False)

    xt_d = nc.dram_tensor("xt", (128, KN, T), bf, kind="ExternalInput")
    wq_d = nc.dram_tensor("wq", (128, KN, DHL), bf, kind="ExternalInput")
    wk_d = nc.dram_tensor("wk", (128, KN, DHL), bf, kind="ExternalInput")
    wv_d = nc.dram_tensor("wv", (128, KN, DHL), bf, kind="ExternalInput")
    wg_d = nc.dram_tensor("wg", (128, 2 * dh), bf, kind="ExternalInput")
    wo_d = nc.dram_tensor("wo", (128, KO, D), bf, kind="ExternalInput")
    mask_d = nc.dram_tensor("mask", (128, 2, 128), bf, kind="ExternalInput")
    ones_d = nc.dram_tensor("ones", (128, TT), bf, kind="ExternalInput")
    y_d = nc.dram_tensor("y", (T, D), bf, kind="ExternalOutput")

    with ExitStack() as ctx:
        tc = ctx.enter_context(tile.TileContext(nc))
        sb_w = ctx.enter_context(tc.tile_pool(name="wts", bufs=1))
        sb_big = ctx.enter_context(tc.tile_pool(name="big", bufs=1))
        sb_e = ctx.enter_context(tc.tile_pool(name="e", bufs=4))
        sb_sig = ctx.enter_context(tc.tile_pool(name="sig", bufs=2))
        sb_nrm = ctx.enter_context(tc.tile_pool(name="nrm", bufs=2))
        sb_y = ctx.enter_context(tc.tile_pool(name="ysb", bufs=3))
        ps_sc = ctx.enter_context(
            tc.tile_pool(name="pssc", bufs=2, space=bass.MemorySpace.PSUM))
        ps_f = ctx.enter_context(
            tc.tile_pool(name="psf", bufs=1, space=bass.MemorySpace.PSUM))
        ps_u = ctx.enter_context(
            tc.tile_pool(name="psu", bufs=1, space=bass.MemorySpace.PSUM))

        # ---- persistent SBUF tensors ----
        xt = sb_big.tile([128, KN, T], bf, tag="xt")
        wq = sb_w.tile([128, KN, DHL], bf, tag="wq")
        wk = sb_w.tile([128, KN, DHL], bf, tag="wk")
        wv = sb_w.tile([128, KN, DHL], bf, tag="wv")
        wg = sb_w.tile([128, 2 * dh], bf, tag="wg")
        wo = sb_w.tile([128, KO, D], bf, tag="wo")
        msk = sb_w.tile([128, 2, 128], bf, tag="msk")
        qt = [sb_big.tile([128, T], bf, tag=f"qt{p}", name=f"qt{p}")
              for p in range(NP)]
        kt = [sb_big.tile([128, T], bf, tag=f"kt{p}", name=f"kt{p}")
              for p in range(NP)]
        ot = [sb_big.tile([128, T], bf, tag=f"ot{p}", name=f"ot{p}")
              for p in range(NP)]
        # vg head index is jp = 2*j + p  (j = row-group half, p = pair)
        vg = sb_big.tile([128, TT, h_loc, VGW], bf, tag="vg")
        # K=1 stationary ones row at partition 64 for the PE den-broadcast
        ones65 = sb_w.tile([65, 64], bf, tag="ones65")
        nc.vector.memset(ones65[64:65, :], 1.0)

        # ---- input DMAs (split across the sync + gpsimd queues) ----
        # sync: wq, even xt chunk-pair-0 k-planes, wk.
        # gpsimd: wv, odd xt planes, gate/mask consts, back half of x, wo.
        nc.sync.dma_start(wq[:], wq_d[:])
        for k in range(0, KN, 2):
            nc.sync.dma_start(xt[:, k, 0:CPW * W], xt_d[:, k, 0:CPW * W])
            nc.gpsimd.dma_start(xt[:, k + 1, 0:CPW * W],
                                xt_d[:, k + 1, 0:CPW * W])
        nc.sync.dma_start(wk[:], wk_d[:])
        nc.gpsimd.dma_start(wv[:], wv_d[:])
        nc.gpsimd.dma_start(wg[:], wg_d[:])
        nc.gpsimd.dma_start(msk[:], mask_d[:])
        for jp in range(h_loc):
            nc.gpsimd.dma_start(vg[:, :, jp, dh], ones_d[:])
        if T > CPW * W:
            nc.gpsimd.dma_start(xt[:, :, CPW * W:T], xt_d[:, :, CPW * W:T])
        nc.gpsimd.dma_start(wo[:], wo_d[:])

        # warm the ACT exp table during the DMA wait (first real exp
        # otherwise pays a ~1.5us ACT_TABLE_LOAD mid-phase-B)
        warm = sb_w.tile([1, 8], f32, tag="warm")
        nc.vector.memset(warm[:], 0.0)
        nc.scalar.activation(warm[:], warm[:], AF.Exp)

        # ---- phase-A jobs ----
        def qk_job(w_sb, dst, p, c0, ncc, pool):
            # ncc consecutive chunks per job; the chunks share each k's
            # LDWEIGHTS (consecutive same-lhsT matmuls are elided by
            # --enable-ldw-opt=true)
            ps = pool.tile([128, 2, W], f32, tag=pool._qk_tag)
            for k in range(KN):
                for cc in range(ncc):
                    nc.tensor.matmul(
                        ps[:, cc, :], w_sb[:, k, 128 * p:128 * p + 128],
                        xt[:, k, (c0 + cc) * W:(c0 + cc + 1) * W],
                        start=(k == 0), stop=(k == KN - 1),
                        skip_group_check=True)
            nc.vector.tensor_copy(
                dst[:, c0 * W:(c0 + ncc) * W]
                .rearrange("p (a b) -> p a b", a=ncc),
                ps[:, 0:ncc, :])

        def vg_job(ti, pool):
            # one 2-bank psum tile: bank0 = V (cols 0:DHL, jp-ordered) +
            # j=0 gates (cols DHL:DHL+128); bank1 = j=1 gates (cols 0:128).
            vps = pool.tile([128, 2, W], f32, tag=pool._qk_tag)
            for k in range(KN):
                nc.tensor.matmul(
                    vps[:, 0, 0:DHL],
                    xt[:, k, 128 * ti:128 * ti + 128],
                    wv[:, k, :],
                    start=(k == 0), stop=(k == KN - 1),
                    skip_group_check=True)
            for p in range(NP):
                nc.tensor.matmul(
                    vps[:, 0, DHL + 64 * p:DHL + 64 * p + 64],
                    qt[p][0:64, 128 * ti:128 * ti + 128],
                    wg[0:64, dh * p:dh * p + dh],
                    start=True, stop=True, skip_group_check=True)
            for p in range(NP):
                nc.tensor.matmul(
                    vps[:, 1, 64 * p:64 * p + 64],
                    qt[p][64:128, 128 * ti:128 * ti + 128],
                    wg[64:128, dh * p:dh * p + dh],
                    start=True, stop=True, skip_group_check=True)
            # sigmoid(x) = 0.5*tanh(x/2) + 0.5 (stays in the exp table set)
            # gate psum offsets: (j0,p0)=DHL, (j0,p1)=DHL+64, (j1,p0)=W,
            # (j1,p1)=W+64 -> uniform [j: stride W-?]... flat view:
            # base DHL, j stride (W + 0) ... offsets: 256,320 | 512,576
            sig = sb_sig.tile([128, 2, 2, dh], bf, tag="sig")
            g_in = (vps[:].rearrange("p a b -> p (a b)")[:, DHL:DHL + 512]
                    .rearrange("p (j x) -> p j x", j=2)[:, :, 0:128])
            nc.scalar.activation(
                sig[:].rearrange("p j pp d -> p j (pp d)"),
                g_in, AF.Tanh, scale=0.5)
            nc.vector.tensor_scalar(sig[:], sig[:], 0.5, 0.5,
                                    ALU.mult, ALU.add)
            nc.vector.tensor_mul(
                vg[:, ti, :, 0:dh],
                vps[:, 0, 0:DHL].rearrange("p (h d) -> p h d", h=h_loc),
                sig[:].rearrange("p j pp d -> p (j pp) d"))

        # ---- phase-B inner iteration (software-pipelined) ----
        def b_scores_exp(c, p, i, pool):
            base = c * W128
            off = 128 * (i - base) if i >= base else 0
            sc = pool.tile([128, 2, W], f32, tag=pool._qk_tag)
            for j in range(2):
                nc.tensor.matmul(
                    sc[:, j, off:W],
                    kt[p][64 * j:64 * j + 64, 128 * i:128 * i + 128],
                    qt[p][64 * j:64 * j + 64, c * W + off:(c + 1) * W],
                    start=True, stop=True)
            e = sb_e.tile([128, 2, W], bf, tag="e")
            # one merged exp over both banks (measured cheaper than 2
            # single-bank exps: ~1.23us vs 2x0.73us)
            nc.scalar.activation(
                e[:, :, off:W], sc[:, :, off:W], AF.Exp, scale=SCALE)
            if i >= base:
                nc.vector.tensor_mul(e[:, :, off:off + 128],
                                     e[:, :, off:off + 128], msk[:])
            return e, off

        def b_pv(p, i, U2, S, e, off):
            for j in range(2):
                nc.tensor.matmul(
                    U2[0:65, j, off:W],
                    vg[:, i, 2 * j + p, 0:65],
                    e[:, j, off:W],
                    start=(i == 0), stop=(i == S - 1),
                    skip_group_check=True)

        def normalize(c, p, U2):
            # Drain U raw to SBUF first so the single-buffered U psum frees
            # after one DVE copy instead of after the whole den chain.
            uns = sb_nrm.tile([65, 2, W], bf, tag="uns")
            nc.vector.tensor_copy(uns[:], U2[:])
            # Broadcast the denominator row (partition 64) across partitions
            # 0..63 with a K=1 matmul, then reciprocal straight off PSUM.
            bcp = ps_sc.tile([128, 2, W], f32, tag="sc")
            for j in range(2):
                nc.tensor.matmul(
                    bcp[0:64, j, :], ones65[64:65, :],
                    uns[64:65, j, :],
                    start=True, stop=True, skip_group_check=True)
            rec = sb_nrm.tile([64, 2, W], f32, tag="rec64")
            nc.vector.reciprocal_approx_fast(rec[:], bcp[0:64, :, :])
            nc.vector.tensor_mul(ot[p][0:64, c * W:(c + 1) * W],
                                 uns[0:64, 0, :], rec[:, 0, :])
            obB = sb_nrm.tile([64, W], bf, tag="obB")
            nc.vector.tensor_mul(obB[:], uns[0:64, 1, :], rec[:, 1, :])
            nc.sync.dma_start(ot[p][64:128, c * W:(c + 1) * W], obB[:])

        # ---- phase-C job (one 128-token tile, both 512-col slabs) ----
        def c_job(tt, pool, dmaq):
            yp = pool.tile([128, 2, W], f32, tag=pool._qk_tag)
            for kt_i in range(KO):
                for n2 in range(2):
                    nc.tensor.matmul(
                        yp[:, n2, :],
                        ot[kt_i][:, 128 * tt:128 * tt + 128],
                        wo[:, kt_i, n2 * W:(n2 + 1) * W],
                        start=(kt_i == 0), stop=(kt_i == KO - 1),
                        skip_group_check=True)
            ysb = sb_y.tile([128, 2, W], bf, tag="ysb")
            nc.vector.tensor_copy(ysb[:], yp[:])
            dmaq.dma_start(
                y_d[128 * tt:128 * tt + 128, :]
                .rearrange("p (a b) -> p a b", a=2),
                ysb[:])

        ps_sc._qk_tag = "sc"
        ps_f._qk_tag = "f"

        # ---- emission schedule ----
        fillers = deque()

        # upfront: chunk-pair 0 projections + chunk-0 V/gates, alternating
        # between the two psum pools for pipelining.
        up_pools = [ps_sc, ps_f]
        upfront = [
            lambda pl: qk_job(wq, qt[0], 0, 0, CPW, pl),
            lambda pl: qk_job(wk, kt[0], 0, 0, CPW, pl),
            lambda pl: qk_job(wq, qt[1], 1, 0, CPW, pl),
            lambda pl: qk_job(wk, kt[1], 1, 0, CPW, pl),
            lambda pl: vg_job(0, pl),
            lambda pl: vg_job(1, pl),
            lambda pl: vg_job(2, pl),
            lambda pl: vg_job(3, pl),
        ]
        for idx, job in enumerate(upfront):
            job(up_pools[idx % 2])

        # fillers carry a deadline: the global b_iter index before which the
        # job MUST be emitted (its output is consumed by that iteration).
        # Jobs are popped when their deadline approaches (margin below), and
        # otherwise paced evenly so the late (big) chunks still get PE work
        # to hide exp latency.
        start_of = [0] * (NCH + 1)
        for c in range(NCH):
            start_of[c + 1] = start_of[c] + (c + 1) * W128 * NP
        total_iters = start_of[NCH]
        MARGIN = 3

        fillers = []  # list of (deadline, job)
        for c in range(CPW, NCH):
            for job in (lambda c=c: qk_job(wq, qt[0], 0, c, 1, ps_f),
                        lambda c=c: qk_job(wq, qt[1], 1, c, 1, ps_f),
                        lambda c=c: qk_job(wk, kt[0], 0, c, 1, ps_f),
                        lambda c=c: qk_job(wk, kt[1], 1, c, 1, ps_f)):
                fillers.append([start_of[c], job])
        for ti in range(W128, TT):
            c1 = ti // W128
            # first consumer is PV(c1, p=0, i=ti), emitted at in-chunk
            # scores-iteration ti+LAG: spread the vg jobs up to then
            fillers.append([start_of[c1] + ti,
                            lambda ti=ti: vg_job(ti, ps_f)])
        fillers.sort(key=lambda f: f[0])

        n_filler_est = len(fillers) + NCH * W128  # + c_jobs appended later
        pace = n_filler_est / max(1, total_iters)
        budget = 0.0
        giter = 0

        def pop_fillers():
            nonlocal budget
            while fillers and fillers[0][0] <= giter + MARGIN:
                fillers.pop(0)[1]()
                budget -= 1.0
            while budget >= 1.0 and fillers:
                fillers.pop(0)[1]()
                budget -= 1.0

        LAG = 2
        for c in range(NCH):
            S = (c + 1) * W128
            for p in range(NP):
                U2 = ps_u.tile([65, 2, W], f32, tag="U", name="U2")
                pend = deque()
                rot = 0
                for i in range(S):
                    # software pipeline with a 2-iteration PV lag: by the
                    # time PV(i-2) enters the in-order PE queue, exp(i-2)
                    # has long finished, so the queue never blocks on ACT.
                    # When no fillers need the f psum bank, rotate scores
                    # through it too (3 slots -> slot recycle never waits
                    # on the exp in flight).
                    if not fillers and rot % 3 == 2:
                        pool = ps_f
                    else:
                        pool = ps_sc
                    rot += 1
                    e, off = b_scores_exp(c, p, i, pool)
                    giter += 1
                    budget += pace
                    pop_fillers()
                    if len(pend) >= LAG:
                        pi, pe_, poff = pend.popleft()
                        b_pv(p, pi, U2, S, pe_, poff)
                    pend.append((i, e, off))
                while pend:
                    pi, pe_, poff = pend.popleft()
                    b_pv(p, pi, U2, S, pe_, poff)
                normalize(c, p, U2)
            for idx, tt in enumerate(range(c * W128, (c + 1) * W128)):
                if c == NCH - 1:
                    # final chunk's out-proj runs after all b_iters: free to
                    # alternate pools and DMA queues for a pipelined tail.
                    fillers.append(
                        [10 ** 9, lambda tt=tt, idx=idx: c_job(
                            tt, [ps_f, ps_sc][idx % 2],
                            [nc.sync, nc.gpsimd][idx % 2])])
                else:
                    # spread over the first half of the next chunk; after
                    # that the deque empties and the scores rotation takes
                    # over the f psum bank
                    fillers.append(
                        [start_of[c + 1] + 2 + idx * W128,
                         lambda tt=tt, idx=idx: c_job(
                            tt, ps_f, [nc.sync, nc.gpsimd][idx % 2])])
                    fillers.sort(key=lambda f: f[0])
        while fillers:
            fillers.pop(0)[1]()

    nc.compile()
    meta = dict(T=T, D=D, h_loc=h_loc, dh=dh, W=W)
    return nc, meta


def _to_bf16(a):
    import ml_dtypes
    return np.asarray(a, dtype=np.float32).astype(ml_dtypes.bfloat16)


def prepare_core_inputs(x, W_qkv, b_qkv, W_g, W_out, b_out,
                        T=T_FULL, D=D_MODEL, h_loc=H_LOC, dh=D_HEAD):
    """Host-side sharding: returns list of per-core input dicts (bf16)."""
    x = np.asarray(x, dtype=np.float32)
    W_qkv = np.asarray(W_qkv, dtype=np.float32)
    W_g = np.asarray(W_g, dtype=np.float32)
    W_out = np.asarray(W_out, dtype=np.float32)
    KN = D // 128
    DHL = h_loc * dh
    KO = DHL // 128
    NP = h_loc // 2
    n_groups = N_CORES // B
    mask1 = (np.arange(128)[:, None] <= np.arange(128)[None, :]).astype(
        np.float32)
    mask = np.ascontiguousarray(
        np.broadcast_to(mask1[:, None, :], (128, 2, 128)))
    # jp order: jp = 2*j + p  ->  head h = 2*p + j
    jp_heads = [2 * (m % NP) + (m // NP) for m in range(h_loc)]

    in_maps = []
    for core in range(N_CORES):
        b, g = divmod(core, n_groups)
        cols = slice(DHL * g, DHL * (g + 1))
        xt = np.ascontiguousarray(
            x[b].T.reshape(KN, 128, T).transpose(1, 0, 2))
        wq = np.ascontiguousarray(
            W_qkv[:, 0 * D:1 * D][:, cols].reshape(KN, 128, DHL)
            .transpose(1, 0, 2))
        wk = np.ascontiguousarray(
            W_qkv[:, 1 * D:2 * D][:, cols].reshape(KN, 128, DHL)
            .transpose(1, 0, 2))
        wv_cols = W_qkv[:, 2 * D:3 * D][:, cols]
        wv_r = np.concatenate(
            [wv_cols[:, dh * h:dh * h + dh] for h in jp_heads], axis=1)
        wv = np.ascontiguousarray(
            wv_r.reshape(KN, 128, DHL).transpose(1, 0, 2))
        wgh = np.zeros((128, 2 * dh), dtype=np.float32)
        for j in range(2):
            for p in range(NP):
                wgh[64 * j:64 * j + 64, dh * p:dh * p + dh] = \
                    W_g[h_loc * g + 2 * p + j]
        wo = np.ascontiguousarray(
            W_out[DHL * g:DHL * (g + 1), :].reshape(KO, 128, D)
            .transpose(1, 0, 2))
        in_maps.append({
            "xt": _to_bf16(xt), "wq": _to_bf16(wq), "wk": _to_bf16(wk),
            "wv": _to_bf16(wv), "wg": _to_bf16(wgh), "wo": _to_bf16(wo),
            "mask": _to_bf16(mask),
            "ones": _to_bf16(np.ones((128, T // 128), dtype=np.float32)),
        })
    return in_maps


def gather_output(results, b_out):
    """Sum the per-core partial projections into the full output."""
    n_groups = N_CORES // B
    b_out = np.asarray(b_out, dtype=np.float32)
    outs = []
    for b in range(B):
        acc = None
        for g in range(n_groups):
            part = np.asarray(results[b * n_groups + g]["y"],
                              dtype=np.float32)
            acc = part.copy() if acc is None else acc + part
        outs.append(acc + b_out[None, :])
    return np.stack(outs, axis=0)


_BUILD_CACHE = {}


def _get_nc():
    key = (T_FULL, D_MODEL, H_LOC, D_HEAD)
    if key not in _BUILD_CACHE:
        _BUILD_CACHE[key] = build_nc()
    return _BUILD_CACHE[key]


def kernel(x, W_qkv, b_qkv, W_g, W_out, b_out):
    # NOTE: do NOT enable --enable-ldw-opt with bf16 weights: walrus
    # codegen crashes in visitInstLdweights (FWL + elision conflict).
    from concourse.bass_utils import run_bass_kernel_spmd

    b_qkv = np.asarray(b_qkv, dtype=np.float32)
    assert not np.any(b_qkv), "nonzero b_qkv not supported by this build"
    nc, _ = _get_nc()
    in_maps = prepare_core_inputs(x, W_qkv, b_qkv, W_g, W_out, b_out)
    res = run_bass_kernel_spmd(nc, in_maps, core_ids=list(range(N_CORES)))
    return gather_output(res.results, b_out).astype(np.float32)


# revision 40
# speedup vs baseline: 1.1329x; 1.1329x over previous
"""Bass/Trainium2 kernel for BilinearlyModulatedAttention (v2, bf16).

Sharding: 8 cores = 2 (batch) x 4 (head groups of 4 heads).
Each core computes, for its batch b and heads [4g, 4g+4):
  QT/KT (feature-major, d x T), V (token-major), bilinear gate, causal
  softmax in transposed layout (scores[s, t]), PV with a ones-column
  giving softmax denominators, normalization, and a partial output
  projection Y_partial = O^T.T @ W_out[rows]. Host sums the 4 partials
  per batch and adds b_out.

v2 changes vs the fp32r baseline (283-297us):
 - ALL matmul operands in bf16 (fp32r measured ~2 cycles/row on HW at
   N=512 warm: 428ns avg; bf16 streams 1 cycle/row and enables FWL
   fast weight loads). PSUM accumulation stays fp32.
 - scores for the two heads of a pair go into ONE 2-bank psum tile
   ([128,2,512]: bank0 = row-group-0 head, bank1 = row-group-64 head,
   satisfying the one-row-group-base-per-bank rule), so exp is ONE
   ACT instruction over both heads (halves ACT instruction count).
 - input DMAs coalesced (~20 instead of ~40) and split across the
   sync + gpsimd queues; x arrives per-k chunk-pair so the first
   matmul starts ~3us in (baseline was DMA-starved for 40us).
 - psum: scores pool [128,2,512]x2 (4 banks) reserved for b_iters,
   filler pool x1 (2 banks) for qk/vg/out-proj jobs, U pool [65,2,512]
   x1 (2 banks). 8 banks total.
 - softmax denominator row copies go through ACT (Copy) instead of DVE;
   normalize DMA bounces ride the gpsimd queue.
 - out-projection does both 512-col slabs per token tile in one job
   (one DVE drain + one y DMA per 128 tokens).
"""

import sys

if "/opt/trn_rl_repo" not in sys.path:
    sys.path.insert(0, "/opt/trn_rl_repo")

import numpy as np

D_MODEL = 1024
N_HEADS = 16
D_HEAD = 64
B = 2
T_FULL = 2048
N_CORES = 8
H_LOC = N_HEADS // (N_CORES // B)  # 4 heads per core

_LDW_PATCHED = False


def _patch_ldw_opt():
    """Compile walrus with --enable-ldw-opt=true (elides redundant
    LDWEIGHTS reloads). Wraps concourse.bass_utils.run_command."""
    global _LDW_PATCHED
    if _LDW_PATCHED:
        return
    import concourse.bass_utils as BU
    orig = BU.run_command

    def run_patched(argv, **kw):
        argv = [a.replace("--enable-ldw-opt=false", "--enable-ldw-opt=true")
                if isinstance(a, str) else a for a in argv]
        return orig(argv, **kw)

    BU.run_command = run_patched
    _LDW_PATCHED = True


def build_nc(T=T_FULL, D=D_MODEL, h_loc=H_LOC, dh=D_HEAD, W=512):
    """Build the Bass module for one core's shard. Returns (nc, meta)."""
    import concourse.bass as bass
    import concourse.mybir as mybir
    import concourse.tile as tile
    from concourse import bacc
    from contextlib import ExitStack
    from collections import deque

    f32 = mybir.dt.float32
    bf = mybir.dt.bfloat16
    AF = mybir.ActivationFunctionType
    ALU = mybir.AluOpType

    KN = D // 128             # k-tiles for the qkv projections
    TT = T // 128             # 128-token tiles
    assert T % W == 0 and W == 512
    NCH = T // W              # chunks
    CPW = min(2, NCH)         # chunks per qk/projection job
    NCP = (NCH + CPW - 1) // CPW  # chunk-pair jobs
    W128 = W // 128           # s-tiles per chunk (4)
    DHL = h_loc * dh          # local head dim total (256)
    NP = h_loc // 2           # head pairs
    KO = DHL // 128           # out-proj k-tiles (2)
    VGW = dh + 1              # vg last dim: 64 V cols + ones col
    SCALE = 1.0 / float(np.sqrt(dh))

    nc = bacc.Bacc("TRN2", target_bir_lowering=False, debug=False)

    xt_d = nc.dram_tensor("xt", (128, KN, T), bf, kind="ExternalInput")
    wq_d = nc.dram_tensor("wq", (128, KN, DHL), bf, kind="ExternalInput")
    wk_d = nc.dram_tensor("wk", (128, KN, DHL), bf, kind="ExternalInput")
    wv_d = nc.dram_tensor("wv", (128, KN, DHL), bf, kind="ExternalInput")
    wg_d = nc.dram_tensor("wg", (128, 2 * dh), bf, kind="ExternalInput")
    wo_d = nc.dram_tensor("wo", (128, KO, D), bf, kind="ExternalInput")
    mask_d = nc.dram_tensor("mask", (128, 2, 128), bf, kind="ExternalInput")
    ones_d = nc.dram_tensor("ones", (128, TT), bf, kind="ExternalInput")
    y_d = nc.dram_tensor("y", (T, D), bf, kind="ExternalOutput")

    with ExitStack() as ctx:
        tc = ctx.enter_context(tile.TileContext(nc))
        sb_w = ctx.enter_context(tc.tile_pool(name="wts", bufs=1))
        sb_big = ctx.enter_context(tc.tile_pool(name="big", bufs=1))
        sb_e = ctx.enter_context(tc.tile_pool(name="e", bufs=4))
        sb_sig = ctx.enter_context(tc.tile_pool(name="sig", bufs=2))
        sb_nrm = ctx.enter_context(tc.tile_pool(name="nrm", bufs=2))
        sb_y = ctx.enter_context(tc.tile_pool(name="ysb", bufs=3))
        ps_sc = ctx.enter_context(
            tc.tile_pool(name="pssc", bufs=2, space=bass.MemorySpace.PSUM))
        ps_f = ctx.enter_context(
            tc.tile_pool(name="psf", bufs=1, space=bass.MemorySpace.PSUM))
        ps_u = ctx.enter_context(
            tc.tile_pool(name="psu", bufs=1, space=bass.MemorySpace.PSUM))

        # ---- persistent SBUF tensors ----
        xt = sb_big.tile([128, KN, T], bf, tag="xt")
        wq = sb_w.tile([128, KN, DHL], bf, tag="wq")
        wk = sb_w.tile([128, KN, DHL], bf, tag="wk")
        wv = sb_w.tile([128, KN, DHL], bf, tag="wv")
        wg = sb_w.tile([128, 2 * dh], bf, tag="wg")
        wo = sb_w.tile([128, KO, D], bf, tag="wo")
        msk = sb_w.tile([128, 2, 128], bf, tag="msk")
        qt = [sb_big.tile([128, T], bf, tag=f"qt{p}", name=f"qt{p}")
              for p in range(NP)]
        kt = [sb_big.tile([128, T], bf, tag=f"kt{p}", name=f"kt{p}")
              for p in range(NP)]
        ot = [sb_big.tile([128, T], bf, tag=f"ot{p}", name=f"ot{p}")
              for p in range(NP)]
        # vg head index is jp = 2*j + p  (j = row-group half, p = pair)
        vg = sb_big.tile([128, TT, h_loc, VGW], bf, tag="vg")
        # K=1 stationary ones row at partition 64 for the PE den-broadcast
        ones65 = sb_w.tile([65, 64], bf, tag="ones65")
        nc.vector.memset(ones65[64:65, :], 1.0)

        # ---- input DMAs (split across the sync + gpsimd queues) ----
        # sync: wq, even xt chunk-pair-0 k-planes, wk.
        # gpsimd: wv, odd xt planes, gate/mask consts, back half of x, wo.
        nc.sync.dma_start(wq[:], wq_d[:])
        for k in range(0, KN, 2):
            nc.sync.dma_start(xt[:, k, 0:CPW * W], xt_d[:, k, 0:CPW * W])
            nc.gpsimd.dma_start(xt[:, k + 1, 0:CPW * W],
                                xt_d[:, k + 1, 0:CPW * W])
        nc.sync.dma_start(wk[:], wk_d[:])
        nc.gpsimd.dma_start(wv[:], wv_d[:])
        nc.gpsimd.dma_start(wg[:], wg_d[:])
        nc.gpsimd.dma_start(msk[:], mask_d[:])
        for jp in range(h_loc):
            nc.gpsimd.dma_start(vg[:, :, jp, dh], ones_d[:])
        if T > CPW * W:
            nc.gpsimd.dma_start(xt[:, :, CPW * W:T], xt_d[:, :, CPW * W:T])
        nc.gpsimd.dma_start(wo[:], wo_d[:])

        # warm the ACT exp table during the DMA wait (first real exp
        # otherwise pays a ~1.5us ACT_TABLE_LOAD mid-phase-B)
        warm = sb_w.tile([1, 8], f32, tag="warm")
        nc.vector.memset(warm[:], 0.0)
        nc.scalar.activation(warm[:], warm[:], AF.Exp)

        # ---- phase-A jobs ----
        def qk_job(w_sb, dst, p, c0, ncc, pool):
            # ncc consecutive chunks per job; the chunks share each k's
            # LDWEIGHTS (consecutive same-lhsT matmuls are elided by
            # --enable-ldw-opt=true)
            ps = pool.tile([128, 2, W], f32, tag=pool._qk_tag)
            for k in range(KN):
                for cc in range(ncc):
                    nc.tensor.matmul(
                        ps[:, cc, :], w_sb[:, k, 128 * p:128 * p + 128],
                        xt[:, k, (c0 + cc) * W:(c0 + cc + 1) * W],
                        start=(k == 0), stop=(k == KN - 1),
                        skip_group_check=True)
            nc.vector.tensor_copy(
                dst[:, c0 * W:(c0 + ncc) * W]
                .rearrange("p (a b) -> p a b", a=ncc),
                ps[:, 0:ncc, :])

        def vg_job(ti, pool):
            # one 2-bank psum tile: bank0 = V (cols 0:DHL, jp-ordered) +
            # j=0 gates (cols DHL:DHL+128); bank1 = j=1 gates (cols 0:128).
            vps = pool.tile([128, 2, W], f32, tag=pool._qk_tag)
            for k in range(KN):
                nc.tensor.matmul(
                    vps[:, 0, 0:DHL],
                    xt[:, k, 128 * ti:128 * ti + 128],
                    wv[:, k, :],
                    start=(k == 0), stop=(k == KN - 1),
                    skip_group_check=True)
            for p in range(NP):
                nc.tensor.matmul(
                    vps[:, 0, DHL + 64 * p:DHL + 64 * p + 64],
                    qt[p][0:64, 128 * ti:128 * ti + 128],
                    wg[0:64, dh * p:dh * p + dh],
                    start=True, stop=True, skip_group_check=True)
            for p in range(NP):
                nc.tensor.matmul(
                    vps[:, 1, 64 * p:64 * p + 64],
                    qt[p][64:128, 128 * ti:128 * ti + 128],
                    wg[64:128, dh * p:dh * p + dh],
                    start=True, stop=True, skip_group_check=True)
            # sigmoid(x) = 0.5*tanh(x/2) + 0.5 (stays in the exp table set)
            # gate psum offsets: (j0,p0)=DHL, (j0,p1)=DHL+64, (j1,p0)=W,
            # (j1,p1)=W+64 -> uniform [j: stride W-?]... flat view:
            # base DHL, j stride (W + 0) ... offsets: 256,320 | 512,576
            sig = sb_sig.tile([128, 2, 2, dh], bf, tag="sig")
            g_in = (vps[:].rearrange("p a b -> p (a b)")[:, DHL:DHL + 512]
                    .rearrange("p (j x) -> p j x", j=2)[:, :, 0:128])
            nc.scalar.activation(
                sig[:].rearrange("p j pp d -> p j (pp d)"),
                g_in, AF.Tanh, scale=0.5)
            nc.vector.tensor_scalar(sig[:], sig[:], 0.5, 0.5,
                                    ALU.mult, ALU.add)
            nc.vector.tensor_mul(
                vg[:, ti, :, 0:dh],
                vps[:, 0, 0:DHL].rearrange("p (h d) -> p h d", h=h_loc),
                sig[:].rearrange("p j pp d -> p (j pp) d"))

        # ---- phase-B inner iteration (software-pipelined) ----
        def b_scores_exp(c, p, i, pool):
            base = c * W128
            off = 128 * (i - base) if i >= base else 0
            sc = pool.tile([128, 2, W], f32, tag=pool._qk_tag)
            for j in range(2):
                nc.tensor.matmul(
                    sc[:, j, off:W],
                    kt[p][64 * j:64 * j + 64, 128 * i:128 * i + 128],
                    qt[p][64 * j:64 * j + 64, c * W + off:(c + 1) * W],
                    start=True, stop=True)
            e = sb_e.tile([128, 2, W], bf, tag="e")
            # one merged exp over both banks (measured cheaper than 2
            # single-bank exps: ~1.23us vs 2x0.73us)
            nc.scalar.activation(
                e[:, :, off:W], sc[:, :, off:W], AF.Exp, scale=SCALE)
            if i >= base:
                # diagonal mask on gpsimd: slower per-op but off the DVE
                # queue, and the 2-iteration PV lag hides its latency
                nc.gpsimd.tensor_mul(e[:, :, off:off + 128],
                                     e[:, :, off:off + 128], msk[:])
            return e, off

        def b_pv(p, i, U2, S, e, off):
            for j in range(2):
                nc.tensor.matmul(
                    U2[0:65, j, off:W],
                    vg[:, i, 2 * j + p, 0:65],
                    e[:, j, off:W],
                    start=(i == 0), stop=(i == S - 1),
                    skip_group_check=True)

        def normalize_start(c, p, U2):
            # Drain U raw to SBUF so the single-buffered U psum frees after
            # one DVE copy instead of after the whole den chain.
            uns = sb_nrm.tile([65, 2, W], bf, tag="uns")
            nc.vector.tensor_copy(uns[:], U2[:])
            return uns

        def normalize_finish(c, p, uns):
            # Deferred a few iterations into the next stream: the bc matmul
            # sits in the in-order PE queue, so it must not be emitted until
            # the uns copy (DVE, possibly backlogged) has had time to land.
            # Broadcast the denominator row (partition 64) across partitions
            # 0..63 with a K=1 matmul, then reciprocal straight off PSUM.
            bcp = ps_sc.tile([128, 2, W], f32, tag="sc")
            for j in range(2):
                nc.tensor.matmul(
                    bcp[0:64, j, :], ones65[64:65, :],
                    uns[64:65, j, :],
                    start=True, stop=True, skip_group_check=True)
            rec = sb_nrm.tile([64, 2, W], f32, tag="rec64")
            nc.vector.reciprocal_approx_fast(rec[:], bcp[0:64, :, :])
            nc.vector.tensor_mul(ot[p][0:64, c * W:(c + 1) * W],
                                 uns[0:64, 0, :], rec[:, 0, :])
            obB = sb_nrm.tile([64, W], bf, tag="obB")
            nc.vector.tensor_mul(obB[:], uns[0:64, 1, :], rec[:, 1, :])
            nc.sync.dma_start(ot[p][64:128, c * W:(c + 1) * W], obB[:])

        # ---- phase-C job (one 128-token tile, both 512-col slabs) ----
        def c_job(tt, pool, dmaq):
            yp = pool.tile([128, 2, W], f32, tag=pool._qk_tag)
            for kt_i in range(KO):
                for n2 in range(2):
                    nc.tensor.matmul(
                        yp[:, n2, :],
                        ot[kt_i][:, 128 * tt:128 * tt + 128],
                        wo[:, kt_i, n2 * W:(n2 + 1) * W],
                        start=(kt_i == 0), stop=(kt_i == KO - 1),
                        skip_group_check=True)
            ysb = sb_y.tile([128, 2, W], bf, tag="ysb")
            nc.vector.tensor_copy(ysb[:], yp[:])
            dmaq.dma_start(
                y_d[128 * tt:128 * tt + 128, :]
                .rearrange("p (a b) -> p a b", a=2),
                ysb[:])

        ps_sc._qk_tag = "sc"
        ps_f._qk_tag = "f"

        # ---- emission schedule ----
        fillers = deque()

        # upfront: chunk-pair 0 projections + chunk-0 V/gates, alternating
        # between the two psum pools for pipelining.
        up_pools = [ps_sc, ps_f]
        upfront = [
            lambda pl: qk_job(wq, qt[0], 0, 0, CPW, pl),
            lambda pl: qk_job(wk, kt[0], 0, 0, CPW, pl),
            lambda pl: qk_job(wq, qt[1], 1, 0, CPW, pl),
            lambda pl: qk_job(wk, kt[1], 1, 0, CPW, pl),
            lambda pl: vg_job(0, pl),
            lambda pl: vg_job(1, pl),
            lambda pl: vg_job(2, pl),
            lambda pl: vg_job(3, pl),
        ]
        for idx, job in enumerate(upfront):
            job(up_pools[idx % 2])

        # fillers carry a deadline: the global b_iter index before which the
        # job MUST be emitted (its output is consumed by that iteration).
        # Jobs are popped when their deadline approaches (margin below), and
        # otherwise paced evenly so the late (big) chunks still get PE work
        # to hide exp latency.
        start_of = [0] * (NCH + 1)
        for c in range(NCH):
            start_of[c + 1] = start_of[c] + (c + 1) * W128 * NP
        total_iters = start_of[NCH]
        MARGIN = 3

        fillers = []  # list of (deadline, job)
        for c in range(CPW, NCH):
            for job in (lambda c=c: qk_job(wq, qt[0], 0, c, 1, ps_f),
                        lambda c=c: qk_job(wq, qt[1], 1, c, 1, ps_f),
                        lambda c=c: qk_job(wk, kt[0], 0, c, 1, ps_f),
                        lambda c=c: qk_job(wk, kt[1], 1, c, 1, ps_f)):
                fillers.append([start_of[c], job])
        for ti in range(W128, TT):
            c1 = ti // W128
            # first consumer is PV(c1, p=0, i=ti), emitted at in-chunk
            # scores-iteration ti+LAG: spread the vg jobs up to then
            fillers.append([start_of[c1] + ti,
                            lambda ti=ti: vg_job(ti, ps_f)])
        fillers.sort(key=lambda f: f[0])

        n_filler_est = len(fillers) + NCH * W128  # + c_jobs appended later
        pace = n_filler_est / max(1, total_iters)
        budget = 0.0
        giter = 0

        def pop_fillers():
            nonlocal budget
            while fillers and fillers[0][0] <= giter + MARGIN:
                fillers.pop(0)[1]()
                budget -= 1.0
            while budget >= 1.0 and fillers:
                fillers.pop(0)[1]()
                budget -= 1.0

        LAG = 2
        DEFER = 3  # iterations into the next stream before normalize_finish
        pending_norm = None
        for c in range(NCH):
            S = (c + 1) * W128
            for p in range(NP):
                U2 = ps_u.tile([65, 2, W], f32, tag="U", name="U2")
                pend = deque()
                rot = 0
                for i in range(S):
                    # software pipeline with a 2-iteration PV lag: by the
                    # time PV(i-2) enters the in-order PE queue, exp(i-2)
                    # has long finished, so the queue never blocks on ACT.
                    # When no fillers need the f psum bank, rotate scores
                    # through it too (3 slots -> slot recycle never waits
                    # on the exp in flight).
                    if not fillers and rot % 3 == 2:
                        pool = ps_f
                    else:
                        pool = ps_sc
                    rot += 1
                    e, off = b_scores_exp(c, p, i, pool)
                    giter += 1
                    budget += pace
                    if i == DEFER and pending_norm is not None:
                        normalize_finish(*pending_norm)
                        pending_norm = None
                    pop_fillers()
                    if len(pend) >= LAG:
                        pi, pe_, poff = pend.popleft()
                        b_pv(p, pi, U2, S, pe_, poff)
                    pend.append((i, e, off))
                while pend:
                    pi, pe_, poff = pend.popleft()
                    b_pv(p, pi, U2, S, pe_, poff)
                uns = normalize_start(c, p, U2)
                pending_norm = (c, p, uns)
            for idx, tt in enumerate(range(c * W128, (c + 1) * W128)):
                if c == NCH - 1:
                    # final chunk's out-proj runs after all b_iters: free to
                    # alternate pools and DMA queues for a pipelined tail.
                    fillers.append(
                        [10 ** 9, lambda tt=tt, idx=idx: c_job(
                            tt, [ps_f, ps_sc][idx % 2],
                            [nc.sync, nc.gpsimd][idx % 2])])
                else:
                    # spread over the next chunk, but only after the
                    # deferred normalize_finish (the ot producer) lands:
                    # pop happens at in-chunk iter (deadline - MARGIN - 1)
                    fillers.append(
                        [start_of[c + 1] + DEFER + MARGIN + 2 + idx * W128,
                         lambda tt=tt, idx=idx: c_job(
                            tt, ps_f, [nc.sync, nc.gpsimd][idx % 2])])
                    fillers.sort(key=lambda f: f[0])
        normalize_finish(*pending_norm)
        while fillers:
            fillers.pop(0)[1]()

    nc.compile()
    meta = dict(T=T, D=D, h_loc=h_loc, dh=dh, W=W)
    return nc, meta


def _to_bf16(a):
    import ml_dtypes
    return np.asarray(a, dtype=np.float32).astype(ml_dtypes.bfloat16)


def prepare_core_inputs(x, W_qkv, b_qkv, W_g, W_out, b_out,
                        T=T_FULL, D=D_MODEL, h_loc=H_LOC, dh=D_HEAD):
    """Host-side sharding: returns list of per-core input dicts (bf16)."""
    x = np.asarray(x, dtype=np.float32)
    W_qkv = np.asarray(W_qkv, dtype=np.float32)
    W_g = np.asarray(W_g, dtype=np.float32)
    W_out = np.asarray(W_out, dtype=np.float32)
    KN = D // 128
    DHL = h_loc * dh
    KO = DHL // 128
    NP = h_loc // 2
    n_groups = N_CORES // B
    mask1 = (np.arange(128)[:, None] <= np.arange(128)[None, :]).astype(
        np.float32)
    mask = np.ascontiguousarray(
        np.broadcast_to(mask1[:, None, :], (128, 2, 128)))
    # jp order: jp = 2*j + p  ->  head h = 2*p + j
    jp_heads = [2 * (m % NP) + (m // NP) for m in range(h_loc)]

    in_maps = []
    for core in range(N_CORES):
        b, g = divmod(core, n_groups)
        cols = slice(DHL * g, DHL * (g + 1))
        xt = np.ascontiguousarray(
            x[b].T.reshape(KN, 128, T).transpose(1, 0, 2))
        wq = np.ascontiguousarray(
            W_qkv[:, 0 * D:1 * D][:, cols].reshape(KN, 128, DHL)
            .transpose(1, 0, 2))
        wk = np.ascontiguousarray(
            W_qkv[:, 1 * D:2 * D][:, cols].reshape(KN, 128, DHL)
            .transpose(1, 0, 2))
        wv_cols = W_qkv[:, 2 * D:3 * D][:, cols]
        wv_r = np.concatenate(
            [wv_cols[:, dh * h:dh * h + dh] for h in jp_heads], axis=1)
        wv = np.ascontiguousarray(
            wv_r.reshape(KN, 128, DHL).transpose(1, 0, 2))
        wgh = np.zeros((128, 2 * dh), dtype=np.float32)
        for j in range(2):
            for p in range(NP):
                wgh[64 * j:64 * j + 64, dh * p:dh * p + dh] = \
                    W_g[h_loc * g + 2 * p + j]
        wo = np.ascontiguousarray(
            W_out[DHL * g:DHL * (g + 1), :].reshape(KO, 128, D)
            .transpose(1, 0, 2))
        in_maps.append({
            "xt": _to_bf16(xt), "wq": _to_bf16(wq), "wk": _to_bf16(wk),
            "wv": _to_bf16(wv), "wg": _to_bf16(wgh), "wo": _to_bf16(wo),
            "mask": _to_bf16(mask),
            "ones": _to_bf16(np.ones((128, T // 128), dtype=np.float32)),
        })
    return in_maps


def gather_output(results, b_out):
    """Sum the per-core partial projections into the full output."""
    n_groups = N_CORES // B
    b_out = np.asarray(b_out, dtype=np.float32)
    outs = []
    for b in range(B):
        acc = None
        for g in range(n_groups):
            part = np.asarray(results[b * n_groups + g]["y"],
                              dtype=np.float32)
            acc = part.copy() if acc is None else acc + part
        outs.append(acc + b_out[None, :])
    return np.stack(outs, axis=0)


_BUILD_CACHE = {}


def _get_nc():
    key = (T_FULL, D_MODEL, H_LOC, D_HEAD)
    if key not in _BUILD_CACHE:
        _BUILD_CACHE[key] = build_nc()
    return _BUILD_CACHE[key]


def kernel(x, W_qkv, b_qkv, W_g, W_out, b_out):
    # NOTE: do NOT enable --enable-ldw-opt with bf16 weights: walrus
    # codegen crashes in visitInstLdweights (FWL + elision conflict).
    from concourse.bass_utils import run_bass_kernel_spmd

    b_qkv = np.asarray(b_qkv, dtype=np.float32)
    assert not np.any(b_qkv), "nonzero b_qkv not supported by this build"
    nc, _ = _get_nc()
    in_maps = prepare_core_inputs(x, W_qkv, b_qkv, W_g, W_out, b_out)
    res = run_bass_kernel_spmd(nc, in_maps, core_ids=list(range(N_CORES)))
    return gather_output(res.results, b_out).astype(np.float32)
